# revision 31
# baseline (speedup 1.0000x reference)
"""Multi-head attention (B=4, S=2048, D=1024, H=16, Dh=64) on 8 TRN2 NeuronCores.

Sharding: core = (batch, head_group): 4 batches x 2 head-groups of 8 heads.
Fully data-parallel SPMD - no collectives.

v18: ScalarE-exp is the hard floor (~264us busy); v17 measured 425us because
the exp stream starved ~105us waiting on front-loaded projection fillers.
Changes vs v17:
  - Step order is (quad OUTER, qb, kt): quad 0 only needs pair-0/1 halves of
    the K/V/Q projections, so only half the projection mass is prerequisite
    to the first 64 steps; pair-2/3 projections run as fillers during quad 0.
  - V projections split into per-quad half chunks (N=256).
  - Minimal phase 1: only K(block0, p0/p1) + Q0(p0/p1) + v_t[0] half-0 are
    computed before the stream starts (first exp ~15us instead of ~34us).
  - K/Q staged twice (pass 1 for p01, re-DMA pass 2 for p23) so stage rings
    stay small; epool deepened to 16 so AV/V work can lag the exp stream.
All matmul operands bf16; masking via host-zeroed V row + masked ones vector
in the softmax-sum matmul; host zeroes the Q_len row of the output.
"""

from contextlib import ExitStack

import ml_dtypes
import numpy as np

import concourse.bass as bass
import concourse.bacc as bacc
import concourse.mybir as mybir
import concourse.tile as tile
from concourse.bass_utils import run_bass_kernel_spmd
from concourse.masks import make_identity

B = 4
SEQ = 2048
DM = 1024
H = 16
DH = 64
NCORES = 8
CPC = 512          # output columns per core (8 heads x 64)
P = 128
NQB = SEQ // 512   # q blocks of 512
NKT = SEQ // P     # k tiles of 128
NDT = DM // P      # d_model tiles of 128

F32 = mybir.dt.float32
BF16 = mybir.dt.bfloat16
EXP = mybir.ActivationFunctionType.Exp

_compiled = None


def _emit(ctx: ExitStack, tc: tile.TileContext, qt, kt, vt, wq, wk, wv, bmask, out):
    nc = tc.nc

    small = ctx.enter_context(tc.tile_pool(name="small", bufs=1))
    wpool = ctx.enter_context(tc.tile_pool(name="wpool", bufs=1))
    kstg = ctx.enter_context(tc.tile_pool(name="kstg", bufs=4))
    vstg = ctx.enter_context(tc.tile_pool(name="vstg", bufs=4))
    qstg = ctx.enter_context(tc.tile_pool(name="qstg", bufs=2))
    proj = ctx.enter_context(tc.tile_pool(name="proj", bufs=1))
    epool = ctx.enter_context(tc.tile_pool(name="epool", bufs=16))
    opool = ctx.enter_context(tc.tile_pool(name="opool", bufs=2))
    oparts = ctx.enter_context(tc.tile_pool(name="oparts", bufs=2))
    ps_sc = ctx.enter_context(tc.tile_pool(name="ps_sc", bufs=2, space="PSUM"))
    ps_ot = ctx.enter_context(tc.tile_pool(name="ps_ot", bufs=2, space="PSUM"))
    ps_sm = ctx.enter_context(tc.tile_pool(name="ps_sm", bufs=1, space="PSUM"))
    ps_aux = ctx.enter_context(tc.tile_pool(name="ps_aux", bufs=1, space="PSUM"))

    ident = small.tile([P, P], F32)
    make_identity(nc, ident[:])
    ident_bf = small.tile([P, P], BF16)
    nc.vector.tensor_copy(ident_bf[:], ident[:])
    mones_f = small.tile([P, NKT], F32)
    nc.sync.dma_start(mones_f[:], bmask.ap())
    mones = small.tile([P, NKT], BF16)
    nc.vector.tensor_copy(mones[:], mones_f[:])
    ones_col = small.tile([P, 1], BF16)
    nc.vector.memset(ones_col[:], 1.0)

    # warm the PE (HAM un-throttle needs ~3.4us of activity) while the
    # phase-1 DMAs stream; results are never read.  64 MMs cover ~5.5us so
    # the PE never sees a full 3.4us idle window before the first chunk.
    warm_ps = ps_aux.tile([P, 512], F32, tag="aux", name="warm_ps")
    for i in range(64):
        nc.tensor.matmul(warm_ps[:, 0:128], ident_bf[:], ident_bf[:],
                         start=(i == 0), stop=(i == 63))

    kt_r = kt.ap()
    vt_r = vt.ap()
    qt_r = qt.ap()

    w_sb = {}

    def load_w(name, w, half, eng):
        # half 0 = output cols 0:256 (pairs 0/1), half 1 = cols 256:512;
        # host supplies [P, 2, NDT, 256] so each partition line is one
        # contiguous 4KB transfer
        t = wpool.tile([P, NDT, CPC // 2], BF16, tag=f"{name}{half}",
                       name=f"{name}{half}")
        eng.dma_start(t[:], w.ap()[:, half])
        w_sb[(name, half)] = t

    kproj = [proj.tile([P, SEQ], BF16, tag=f"kproj{p}", name=f"kproj{p}")
             for p in range(4)]
    qproj = [[proj.tile([P, 512], BF16, tag=f"qproj{p}_{qb}", name=f"qproj{p}_{qb}")
              for qb in range(NQB)] for p in range(4)]
    v_t = [proj.tile([P, 512], BF16, tag=f"v{k}", name=f"v{k}") for k in range(NKT)]

    def stage_block(src_r, blk, pool, tg, nm, eng):
        # host supplies [P, 4, NDT, 512]: one contiguous 8KB line/partition
        st = pool.tile([P, NDT, 512], BF16, tag=tg, name=f"st_{nm}")
        eng.dma_start(st[:], src_r[:, blk])
        return st

    def kq_chunk(wname, st, dst, p, pool):
        ps = pool.tile([P, 512], F32, tag="scores" if pool is ps_sc else "aux")
        for dt in range(NDT):
            nc.tensor.matmul(
                ps[:],
                w_sb[(wname, p // 2)][:, dt, 128 * (p % 2):128 * (p % 2 + 1)],
                st[:, dt, :],
                start=(dt == 0),
                stop=(dt == NDT - 1),
            )
        nc.vector.tensor_copy(dst[:], ps[:])

    def kq_chunk_half(wname, st, dst, p, ch):
        # ~0.87us filler quantum: half the seq columns of a [128,512] chunk
        ps = ps_aux.tile([P, 512], F32, tag="aux", name=f"kqh_{wname}{p}_{ch}")
        for dt in range(NDT):
            nc.tensor.matmul(
                ps[:, 0:256],
                w_sb[(wname, p // 2)][:, dt, 128 * (p % 2):128 * (p % 2 + 1)],
                st[:, dt, 256 * ch:256 * (ch + 1)],
                start=(dt == 0),
                stop=(dt == NDT - 1),
            )
        nc.vector.tensor_copy(dst[:, 256 * ch:256 * (ch + 1)], ps[:, 0:256])

    def v_chunk_half(st, kt_i, half):
        sub = kt_i % 4
        ps = ps_aux.tile([P, 512], F32, tag="aux", name=f"vh{kt_i}_{half}")
        for dt in range(NDT):
            nc.tensor.matmul(
                ps[:, 0:256],
                st[:, dt, 128 * sub:128 * (sub + 1)],
                w_sb[("wv", half)][:, dt, :],
                start=(dt == 0),
                stop=(dt == NDT - 1),
            )
        nc.vector.tensor_copy(
            v_t[kt_i][:, 256 * half:256 * (half + 1)], ps[:, 0:256]
        )

    # ---- attention stream helpers ---------------------------------------
    quad_state = {}
    pend = {}

    def emit_scores(step):
        qb, quad, kt_i = step
        pairs = (2 * quad, 2 * quad + 1)
        e_tiles = []
        for pi, pr in enumerate(pairs):
            st_ps = ps_sc.tile([P, 1024], F32, tag="scores")
            for hh in range(2):
                rows = slice(64 * hh, 64 * (hh + 1))
                nc.tensor.matmul(
                    st_ps[:, 512 * hh:512 * (hh + 1)],
                    kproj[pr][rows, kt_i * P:(kt_i + 1) * P],
                    qproj[pr][qb][rows, :],
                    start=True,
                    stop=True,
                    tile_position=(64 * hh, 0),
                )
            e = epool.tile([P, 1024], BF16, tag="e")
            nc.scalar.activation(e[:], st_ps[:], EXP, scale=0.125)
            e_tiles.append(e)
        pend[step] = e_tiles

    def emit_av(step):
        qb, quad, kt_i = step
        pairs = (2 * quad, 2 * quad + 1)
        if kt_i == 0:
            quad_state[(qb, quad)] = (
                [ps_ot.tile([P, 512], F32, tag="ot", name=f"ot{qb}_{quad}_{i}")
                 for i in range(2)],
                ps_sm.tile([P, 512], F32, tag="sums", name=f"sm{qb}_{quad}"),
            )
        ot_ps, sm_ps = quad_state[(qb, quad)]
        e_tiles = pend.pop(step)
        for pi, pr in enumerate(pairs):
            e = e_tiles[pi]
            for hh in range(2):
                cols = slice(128 * pr + 64 * hh, 128 * pr + 64 * (hh + 1))
                nc.tensor.matmul(
                    ot_ps[pi][64 * hh:64 * (hh + 1), :],
                    v_t[kt_i][:, cols],
                    e[:, 512 * hh:512 * (hh + 1)],
                    start=(kt_i == 0),
                    stop=(kt_i == NKT - 1),
                    tile_position=(0, 64 * hh),
                    skip_group_check=(hh == 1),
                )
        for j in range(4):
            nc.tensor.matmul(
                sm_ps[32 * j:32 * j + 1, :],
                mones[:, kt_i:kt_i + 1],
                e_tiles[j // 2][:, 512 * (j % 2):512 * (j % 2 + 1)],
                start=(kt_i == 0),
                stop=(kt_i == NKT - 1),
                tile_position=(0, 32 * j),
                skip_group_check=(j > 0),
            )

    def make_tail(qb, quad):
        ot_ps, sm_ps = quad_state.pop((qb, quad))
        st = {}

        def t0():
            # free sm + ot banks ASAP (DVE copies only)
            sums_sb = opool.tile([P, 512], F32, tag="sums_sb",
                                 name=f"ssb{qb}_{quad}")
            nc.vector.memset(sums_sb[:], 1.0)
            for j in range(4):
                nc.vector.tensor_copy(
                    sums_sb[32 * j:32 * j + 1, :], sm_ps[32 * j:32 * j + 1, :]
                )
            ot_sb = [opool.tile([P, 512], BF16, tag="ot_sb",
                                name=f"otsb{qb}_{quad}_{i}") for i in range(2)]
            for pi in range(2):
                nc.vector.tensor_copy(ot_sb[pi][:], ot_ps[pi][:])
            st["sums_sb"] = sums_sb
            st["ot_sb"] = ot_sb

        def t1():
            rcp = opool.tile([P, 16], F32, tag="rcp", name=f"rcp{qb}_{quad}")
            for c in range(4):
                tr_s = ps_aux.tile([P, P], F32, tag="aux", name=f"trs{qb}_{quad}_{c}")
                nc.tensor.transpose(tr_s[:], st["sums_sb"][:, c * P:(c + 1) * P],
                                    ident[:])
                nc.vector.reciprocal(
                    rcp[:, 4 * c:4 * c + 4],
                    tr_s.rearrange("p (j r) -> p j r", j=4)[:, :, 0],
                )
            st["rcp"] = rcp
            st["o_part"] = oparts.tile(
                [P, 4, 256], F32, tag="opart", name=f"opart{qb}_{quad}"
            )

        def t_pi(pi):
            o_part, rcp = st["o_part"], st["rcp"]
            for c in range(4):
                tr_o = ps_aux.tile([P, P], BF16, tag="aux",
                                   name=f"tro{qb}_{quad}_{pi}_{c}")
                nc.tensor.transpose(tr_o[:], st["ot_sb"][pi][:, c * P:(c + 1) * P],
                                    ident_bf[:])
                for hh in range(2):
                    lh = 2 * pi + hh
                    nc.vector.tensor_scalar(
                        o_part[:, c, 64 * lh:64 * (lh + 1)],
                        tr_o[:, 64 * hh:64 * (hh + 1)],
                        rcp[:, 4 * c + lh:4 * c + lh + 1],
                        None,
                        mybir.AluOpType.mult,
                    )

        def t_out():
            for c in range(4):
                nc.sync.dma_start(
                    out.ap()[
                        qb * 512 + c * P:qb * 512 + (c + 1) * P,
                        quad * 256:(quad + 1) * 256,
                    ],
                    st["o_part"][:, c, :],
                )

        return t0, [t1, lambda: t_pi(0), lambda: t_pi(1), t_out]

    # ---- phase 1: minimal prerequisites for the first exp ---------------
    # One DMA ring (sync), strict priority order: the critical path to the
    # first exp (wk0+st_k0+wq0+st_q0 = 3MB) streams at full bandwidth; the
    # rest of pass 1 queues behind it in need order.
    load_w("wk", wk, 0, nc.sync)
    st_k0 = stage_block(kt_r, 0, kstg, "kst", "k0", nc.sync)
    load_w("wq", wq, 0, nc.sync)
    st_q0 = stage_block(qt_r, 0, qstg, "qst", "q0", nc.sync)
    st_k1 = stage_block(kt_r, 1, kstg, "kst", "k1", nc.sync)
    load_w("wv", wv, 0, nc.sync)
    st_v0 = stage_block(vt_r, 0, vstg, "vst", "v0", nc.sync)
    kq_chunk("wk", st_k0, kproj[0][:, 0:512], 0, ps_sc)
    kq_chunk("wk", st_k0, kproj[1][:, 0:512], 1, ps_sc)
    kq_chunk("wq", st_q0, qproj[0][0][:], 0, ps_sc)
    kq_chunk("wq", st_q0, qproj[1][0][:], 1, ps_sc)
    v_chunk_half(st_v0, 0, 0)
    kstate = {0: st_k0, 1: st_k1}
    vstate = {0: st_v0}
    qstate = {0: st_q0}
    kstate[2] = stage_block(kt_r, 2, kstg, "kst", "k2", nc.sync)
    vstate[1] = stage_block(vt_r, 1, vstg, "vst", "v1", nc.sync)
    kstate[3] = stage_block(kt_r, 3, kstg, "kst", "k3", nc.sync)
    vstate[2] = stage_block(vt_r, 2, vstg, "vst", "v2", nc.sync)
    vstate[3] = stage_block(vt_r, 3, vstg, "vst", "v3", nc.sync)
    qstate[1] = stage_block(qt_r, 1, qstg, "qst", "q1", nc.sync)

    # step order: quad OUTER so pair-2/3 projections are not prerequisites
    # for the first 64 steps
    steps = [(qb, quad, k) for quad in (0, 1) for qb in range(NQB)
             for k in range(NKT)]

    # ---- filler schedule (keyed by global step index) -------------------
    fillers = {}

    def add(i, fn):
        fillers.setdefault(i, []).append(fn)

    def v_stage(kb, nm):
        def f():
            vstate[kb] = stage_block(vt_r, kb, vstg, "vst", nm, nc.sync)
        return f

    def k_stage(kb, nm):
        def f():
            kstate[kb] = stage_block(kt_r, kb, kstg, "kst", nm, nc.sync)
        return f

    def q_stage(qb, nm):
        def f():
            qstate[qb] = stage_block(qt_r, qb, qstg, "qst", nm, nc.sync)
        return f

    def mkv(kt_i, half):
        def f():
            v_chunk_half(vstate[kt_i // 4], kt_i, half)
        return f

    def mkk(kb, p, ch):
        def f():
            kq_chunk_half("wk", kstate[kb],
                          kproj[p][:, kb * 512:(kb + 1) * 512], p, ch)
        return f

    def mkq(qb, p, ch):
        def f():
            kq_chunk_half("wq", qstate[qb], qproj[p][qb][:], p, ch)
        return f

    # Fillers are ~0.87us quanta laid out against just-in-time deadlines:
    # a quantum read by AV(kt) must be emitted at step <= kt-1; one read by
    # scores(kt) of quad q at step <= 64*q + kt - 2 (scores are emitted one
    # step ahead).  Tail pieces land at unit_end + 2/4/6/8, so those steps
    # mostly stay quantum-free.
    # -- pass 1: quad-0 prerequisites (2 quanta/step: structural catch-up) --
    for kt_i in range(1, NKT):
        add(kt_i - 1, mkv(kt_i, 0))
    add(0, mkk(1, 0, 0))
    add(1, mkk(1, 1, 0))
    add(2, mkk(1, 0, 1))
    add(3, mkk(1, 1, 1))
    add(4, mkk(2, 0, 0))
    add(5, mkk(2, 1, 0))
    add(6, mkk(2, 0, 1))
    add(7, mkk(2, 1, 1))
    add(8, mkk(3, 0, 0))
    add(9, mkk(3, 1, 0))
    add(10, mkk(3, 0, 1))
    add(11, mkk(3, 1, 1))
    add(12, mkq(1, 0, 0))
    add(13, mkq(1, 0, 1))
    add(14, mkq(1, 1, 0))
    add(14, mkq(1, 1, 1))
    # -- pass 1 Q for qb2/qb3 --
    add(16, q_stage(2, "q2"))
    add(26, mkq(2, 0, 0))
    add(27, mkq(2, 0, 1))
    add(28, mkq(2, 1, 0))
    add(29, mkq(2, 1, 1))
    add(30, q_stage(3, "q3"))
    add(41, mkq(3, 0, 0))
    add(42, mkq(3, 0, 1))
    add(43, mkq(3, 1, 0))
    add(44, mkq(3, 1, 1))
    # -- pass 2 weight halves + re-staging (DMA only) --
    add(8, lambda: load_w("wk", wk, 1, nc.sync))
    add(18, lambda: load_w("wv", wv, 1, nc.sync))
    add(34, lambda: load_w("wq", wq, 1, nc.sync))
    add(32, v_stage(0, "v0b"))
    add(36, v_stage(1, "v1b"))
    add(40, k_stage(0, "k0b"))
    add(44, k_stage(1, "k1b"))
    add(44, q_stage(0, "q0b"))
    add(46, v_stage(2, "v2b"))
    add(48, k_stage(2, "k2b"))
    add(50, v_stage(3, "v3b"))
    add(52, k_stage(3, "k3b"))
    add(66, q_stage(1, "q1b"))
    add(80, q_stage(2, "q2b"))
    add(96, q_stage(3, "q3b"))
    # -- pass 2 quanta: V half-1 (deadline 63+kt), K p2/p3 (deadline
    #    62 + 4*kb + 2*ch), Q p2/p3 (deadline 63 + 16*qb) --
    add(45, mkv(0, 1))
    add(46, mkv(1, 1))
    add(47, mkv(2, 1))
    add(48, mkv(3, 1))
    add(50, mkv(4, 1))
    add(50, mkq(0, 2, 0))
    add(52, mkv(5, 1))
    add(52, mkq(0, 2, 1))
    add(54, mkv(6, 1))
    add(54, mkq(0, 3, 0))
    add(56, mkv(7, 1))
    add(56, mkq(0, 3, 1))
    add(57, mkk(0, 2, 0))
    add(57, mkv(8, 1))
    add(58, mkk(0, 3, 0))
    add(58, mkv(9, 1))
    add(59, mkk(0, 2, 1))
    add(59, mkv(10, 1))
    add(60, mkk(0, 3, 1))
    add(60, mkv(11, 1))
    add(61, mkk(1, 2, 0))
    add(61, mkv(12, 1))
    add(62, mkk(1, 3, 0))
    add(62, mkv(13, 1))
    add(63, mkk(1, 2, 1))
    add(63, mkv(14, 1))
    add(64, mkk(1, 3, 1))
    add(64, mkv(15, 1))
    add(65, mkk(2, 2, 0))
    add(66, mkk(2, 3, 0))
    add(67, mkk(2, 2, 1))
    add(68, mkk(2, 3, 1))
    add(69, mkk(3, 2, 0))
    add(70, mkk(3, 3, 0))
    add(71, mkk(3, 2, 1))
    add(72, mkk(3, 3, 1))
    add(74, mkq(1, 2, 0))
    add(75, mkq(1, 2, 1))
    add(76, mkq(1, 3, 0))
    add(77, mkq(1, 3, 1))
    add(88, mkq(2, 2, 0))
    add(89, mkq(2, 2, 1))
    add(90, mkq(2, 3, 0))
    add(91, mkq(2, 3, 1))
    add(104, mkq(3, 2, 0))
    add(105, mkq(3, 2, 1))
    add(106, mkq(3, 3, 0))
    add(107, mkq(3, 3, 1))

    # ---- the pipelined stream -------------------------------------------
    emit_scores(steps[0])
    for i, step in enumerate(steps):
        if i + 1 < len(steps):
            emit_scores(steps[i + 1])
        emit_av(step)
        qb, quad, kt_i = step
        if kt_i == NKT - 1:
            t0, rest = make_tail(qb, quad)
            t0()
            if i + 1 < len(steps):
                # spread tail PE bursts (transposes) over 11 steps so the
                # per-step PE load never exceeds the exp budget
                for j, piece in enumerate(rest):
                    add(min(i + 2 + 3 * j, len(steps) - 1), piece)
            else:
                for piece in rest:
                    piece()
        for fn in fillers.get(i, ()):
            fn()


def build():
    global _compiled
    if _compiled is not None:
        return _compiled
    nc = bacc.Bacc("TRN2", target_bir_lowering=False, debug=False)
    qt = nc.dram_tensor("qt", [P, NQB, NDT, 512], BF16, kind="ExternalInput")
    kt = nc.dram_tensor("kt", [P, NQB, NDT, 512], BF16, kind="ExternalInput")
    vt = nc.dram_tensor("vt", [P, NQB, NDT, 512], BF16, kind="ExternalInput")
    wq = nc.dram_tensor("wq", [P, 2, NDT, 256], BF16, kind="ExternalInput")
    wk = nc.dram_tensor("wk", [P, 2, NDT, 256], BF16, kind="ExternalInput")
    wv = nc.dram_tensor("wv", [P, 2, NDT, 256], BF16, kind="ExternalInput")
    bmask = nc.dram_tensor("bmask", [P, NKT], F32, kind="ExternalInput")
    out = nc.dram_tensor("out", [SEQ, CPC], F32, kind="ExternalOutput")
    with tile.TileContext(nc) as tc:
        with ExitStack() as ctx:
            _emit(ctx, tc, qt, kt, vt, wq, wk, wv, bmask, out)
    nc.compile()
    _compiled = nc
    return nc


def _sw_seq(xt):
    # [DM, SEQ] -> [P, NQB, NDT, 512]: dm = dt*128+p, q = blk*512+qq
    return np.ascontiguousarray(
        xt.reshape(NDT, P, NQB, 512).transpose(1, 2, 0, 3)
    )


def _sw_w(w):
    # [DM, CPC] -> [P, 2, NDT, 256]
    return np.ascontiguousarray(
        w.reshape(NDT, P, 2, 256).transpose(1, 2, 0, 3)
    )


def make_in_maps(Q_seq, K_seq, V_seq, V_len, WQ, WK, WV):
    in_maps = []
    for core in range(NCORES):
        b, hg = divmod(core, 2)
        cols = slice(hg * CPC, (hg + 1) * CPC)
        vl = int(V_len[b, 0])
        bm = np.ones((P, NKT), np.float32)
        bm[vl % P, vl // P] = 0.0
        vt_m = np.ascontiguousarray(V_seq[b].T)
        vt_m[:, vl] = 0.0
        bf = ml_dtypes.bfloat16
        in_maps.append(
            {
                "qt": _sw_seq(Q_seq[b].T.astype(bf)),
                "kt": _sw_seq(K_seq[b].T.astype(bf)),
                "vt": _sw_seq(vt_m.astype(bf)),
                "wq": _sw_w(WQ[:, cols].astype(bf)),
                "wk": _sw_w(WK[:, cols].astype(bf)),
                "wv": _sw_w(WV[:, cols].astype(bf)),
                "bmask": bm,
            }
        )
    return in_maps


def kernel(Q_seq, K_seq, V_seq, Q_len, V_len, WQ, WK, WV, _trace=False):
    nc = build()
    in_maps = make_in_maps(Q_seq, K_seq, V_seq, V_len, WQ, WK, WV)
    res = run_bass_kernel_spmd(
        nc, in_maps, core_ids=list(range(NCORES)), trace=_trace
    )
    out = np.empty((B, SEQ, H * DH), np.float32)
    for core in range(NCORES):
        b, hg = divmod(core, 2)
        out[b, :, hg * CPC:(hg + 1) * CPC] = res.results[core]["out"]
    for b in range(B):
        out[b, int(Q_len[b, 0]), :] = 0.0
    if _trace:
        kernel._last_results = res
    return out


# revision 32
# speedup vs baseline: 1.0098x; 1.0098x over previous
"""Multi-head attention (B=4, S=2048, D=1024, H=16, Dh=64) on 8 TRN2 NeuronCores.

Sharding: core = (batch, head_group): 4 batches x 2 head-groups of 8 heads.
Fully data-parallel SPMD - no collectives.

v18: ScalarE-exp is the hard floor (~264us busy); v17 measured 425us because
the exp stream starved ~105us waiting on front-loaded projection fillers.
Changes vs v17:
  - Step order is (quad OUTER, qb, kt): quad 0 only needs pair-0/1 halves of
    the K/V/Q projections, so only half the projection mass is prerequisite
    to the first 64 steps; pair-2/3 projections run as fillers during quad 0.
  - V projections split into per-quad half chunks (N=256).
  - Minimal phase 1: only K(block0, p0/p1) + Q0(p0/p1) + v_t[0] half-0 are
    computed before the stream starts (first exp ~15us instead of ~34us).
  - K/Q staged twice (pass 1 for p01, re-DMA pass 2 for p23) so stage rings
    stay small; epool deepened to 16 so AV/V work can lag the exp stream.
All matmul operands bf16; masking via host-zeroed V row + masked ones vector
in the softmax-sum matmul; host zeroes the Q_len row of the output.
"""

from contextlib import ExitStack

import ml_dtypes
import numpy as np

import concourse.bass as bass
import concourse.bacc as bacc
import concourse.mybir as mybir
import concourse.tile as tile
from concourse.bass_utils import run_bass_kernel_spmd
from concourse.masks import make_identity

B = 4
SEQ = 2048
DM = 1024
H = 16
DH = 64
NCORES = 8
CPC = 512          # output columns per core (8 heads x 64)
P = 128
NQB = SEQ // 512   # q blocks of 512
NKT = SEQ // P     # k tiles of 128
NDT = DM // P      # d_model tiles of 128

F32 = mybir.dt.float32
BF16 = mybir.dt.bfloat16
EXP = mybir.ActivationFunctionType.Exp

_compiled = None


def _emit(ctx: ExitStack, tc: tile.TileContext, qt, kt, vt, wq, wk, wv, bmask, out):
    nc = tc.nc

    small = ctx.enter_context(tc.tile_pool(name="small", bufs=1))
    wpool = ctx.enter_context(tc.tile_pool(name="wpool", bufs=1))
    kstg = ctx.enter_context(tc.tile_pool(name="kstg", bufs=4))
    vstg = ctx.enter_context(tc.tile_pool(name="vstg", bufs=4))
    qstg = ctx.enter_context(tc.tile_pool(name="qstg", bufs=2))
    proj = ctx.enter_context(tc.tile_pool(name="proj", bufs=1))
    epool = ctx.enter_context(tc.tile_pool(name="epool", bufs=16))
    opool = ctx.enter_context(tc.tile_pool(name="opool", bufs=2))
    oparts = ctx.enter_context(tc.tile_pool(name="oparts", bufs=2))
    ps_sc = ctx.enter_context(tc.tile_pool(name="ps_sc", bufs=2, space="PSUM"))
    ps_ot = ctx.enter_context(tc.tile_pool(name="ps_ot", bufs=2, space="PSUM"))
    ps_sm = ctx.enter_context(tc.tile_pool(name="ps_sm", bufs=1, space="PSUM"))
    ps_aux = ctx.enter_context(tc.tile_pool(name="ps_aux", bufs=1, space="PSUM"))

    ident = small.tile([P, P], F32)
    make_identity(nc, ident[:])
    ident_bf = small.tile([P, P], BF16)
    nc.vector.tensor_copy(ident_bf[:], ident[:])
    mones_f = small.tile([P, NKT], F32)
    nc.sync.dma_start(mones_f[:], bmask.ap())
    mones = small.tile([P, NKT], BF16)
    nc.vector.tensor_copy(mones[:], mones_f[:])
    ones_col = small.tile([P, 1], BF16)
    nc.vector.memset(ones_col[:], 1.0)

    # warm the PE (HAM un-throttle needs ~3.4us of activity) while the
    # phase-1 DMAs stream; results are never read.  64 MMs cover ~5.5us so
    # the PE never sees a full 3.4us idle window before the first chunk.
    warm_ps = ps_aux.tile([P, 512], F32, tag="aux", name="warm_ps")
    for i in range(64):
        nc.tensor.matmul(warm_ps[:, 0:128], ident_bf[:], ident_bf[:],
                         start=(i == 0), stop=(i == 63))

    kt_r = kt.ap()
    vt_r = vt.ap()
    qt_r = qt.ap()

    w_sb = {}

    def load_w(name, w, half, eng):
        # half 0 = output cols 0:256 (pairs 0/1), half 1 = cols 256:512;
        # host supplies [P, 2, NDT, 256] so each partition line is one
        # contiguous 4KB transfer
        t = wpool.tile([P, NDT, CPC // 2], BF16, tag=f"{name}{half}",
                       name=f"{name}{half}")
        eng.dma_start(t[:], w.ap()[:, half])
        w_sb[(name, half)] = t

    kproj = [proj.tile([P, SEQ], BF16, tag=f"kproj{p}", name=f"kproj{p}")
             for p in range(4)]
    qproj = [[proj.tile([P, 512], BF16, tag=f"qproj{p}_{qb}", name=f"qproj{p}_{qb}")
              for qb in range(NQB)] for p in range(4)]
    v_t = [proj.tile([P, 512], BF16, tag=f"v{k}", name=f"v{k}") for k in range(NKT)]

    def stage_block(src_r, blk, pool, tg, nm, eng):
        # host supplies [P, 4, NDT, 512]: one contiguous 8KB line/partition
        st = pool.tile([P, NDT, 512], BF16, tag=tg, name=f"st_{nm}")
        eng.dma_start(st[:], src_r[:, blk])
        return st

    def kq_chunk(wname, st, dst, p, pool):
        ps = pool.tile([P, 512], F32, tag="scores" if pool is ps_sc else "aux")
        for dt in range(NDT):
            nc.tensor.matmul(
                ps[:],
                w_sb[(wname, p // 2)][:, dt, 128 * (p % 2):128 * (p % 2 + 1)],
                st[:, dt, :],
                start=(dt == 0),
                stop=(dt == NDT - 1),
            )
        nc.vector.tensor_copy(dst[:], ps[:])

    def kq_chunk_half(wname, st, dst, p, ch):
        # ~0.87us filler quantum: half the seq columns of a [128,512] chunk
        ps = ps_aux.tile([P, 512], F32, tag="aux", name=f"kqh_{wname}{p}_{ch}")
        for dt in range(NDT):
            nc.tensor.matmul(
                ps[:, 0:256],
                w_sb[(wname, p // 2)][:, dt, 128 * (p % 2):128 * (p % 2 + 1)],
                st[:, dt, 256 * ch:256 * (ch + 1)],
                start=(dt == 0),
                stop=(dt == NDT - 1),
            )
        nc.vector.tensor_copy(dst[:, 256 * ch:256 * (ch + 1)], ps[:, 0:256])

    def v_chunk_half(st, kt_i, half):
        sub = kt_i % 4
        ps = ps_aux.tile([P, 512], F32, tag="aux", name=f"vh{kt_i}_{half}")
        for dt in range(NDT):
            nc.tensor.matmul(
                ps[:, 0:256],
                st[:, dt, 128 * sub:128 * (sub + 1)],
                w_sb[("wv", half)][:, dt, :],
                start=(dt == 0),
                stop=(dt == NDT - 1),
            )
        nc.vector.tensor_copy(
            v_t[kt_i][:, 256 * half:256 * (half + 1)], ps[:, 0:256]
        )

    # ---- attention stream helpers ---------------------------------------
    quad_state = {}
    pend = {}

    def emit_scores(step):
        qb, quad, kt_i = step
        pairs = (2 * quad, 2 * quad + 1)
        e_tiles = []
        for pi, pr in enumerate(pairs):
            st_ps = ps_sc.tile([P, 1024], F32, tag="scores")
            for hh in range(2):
                rows = slice(64 * hh, 64 * (hh + 1))
                nc.tensor.matmul(
                    st_ps[:, 512 * hh:512 * (hh + 1)],
                    kproj[pr][rows, kt_i * P:(kt_i + 1) * P],
                    qproj[pr][qb][rows, :],
                    start=True,
                    stop=True,
                    tile_position=(64 * hh, 0),
                )
            e = epool.tile([P, 1024], BF16, tag="e")
            nc.scalar.activation(e[:], st_ps[:], EXP, scale=0.125)
            e_tiles.append(e)
        pend[step] = e_tiles

    def emit_av(step):
        qb, quad, kt_i = step
        pairs = (2 * quad, 2 * quad + 1)
        if kt_i == 0:
            quad_state[(qb, quad)] = (
                [ps_ot.tile([P, 512], F32, tag="ot", name=f"ot{qb}_{quad}_{i}")
                 for i in range(2)],
                ps_sm.tile([P, 512], F32, tag="sums", name=f"sm{qb}_{quad}"),
            )
        ot_ps, sm_ps = quad_state[(qb, quad)]
        e_tiles = pend.pop(step)
        for pi, pr in enumerate(pairs):
            e = e_tiles[pi]
            for hh in range(2):
                cols = slice(128 * pr + 64 * hh, 128 * pr + 64 * (hh + 1))
                nc.tensor.matmul(
                    ot_ps[pi][64 * hh:64 * (hh + 1), :],
                    v_t[kt_i][:, cols],
                    e[:, 512 * hh:512 * (hh + 1)],
                    start=(kt_i == 0),
                    stop=(kt_i == NKT - 1),
                    tile_position=(0, 64 * hh),
                    skip_group_check=(hh == 1),
                )
        for j in range(4):
            nc.tensor.matmul(
                sm_ps[32 * j:32 * j + 1, :],
                mones[:, kt_i:kt_i + 1],
                e_tiles[j // 2][:, 512 * (j % 2):512 * (j % 2 + 1)],
                start=(kt_i == 0),
                stop=(kt_i == NKT - 1),
                tile_position=(0, 32 * j),
                skip_group_check=(j > 0),
            )

    def make_tail(qb, quad):
        ot_ps, sm_ps = quad_state.pop((qb, quad))
        st = {}

        def t0():
            # free sm + ot banks ASAP (DVE copies only)
            sums_sb = opool.tile([P, 512], F32, tag="sums_sb",
                                 name=f"ssb{qb}_{quad}")
            nc.vector.memset(sums_sb[:], 1.0)
            for j in range(4):
                nc.vector.tensor_copy(
                    sums_sb[32 * j:32 * j + 1, :], sm_ps[32 * j:32 * j + 1, :]
                )
            ot_sb = [opool.tile([P, 512], BF16, tag="ot_sb",
                                name=f"otsb{qb}_{quad}_{i}") for i in range(2)]
            for pi in range(2):
                nc.vector.tensor_copy(ot_sb[pi][:], ot_ps[pi][:])
            st["sums_sb"] = sums_sb
            st["ot_sb"] = ot_sb

        def t1():
            rcp = opool.tile([P, 16], F32, tag="rcp", name=f"rcp{qb}_{quad}")
            for c in range(4):
                tr_s = ps_aux.tile([P, P], F32, tag="aux", name=f"trs{qb}_{quad}_{c}")
                nc.tensor.transpose(tr_s[:], st["sums_sb"][:, c * P:(c + 1) * P],
                                    ident[:])
                nc.vector.reciprocal(
                    rcp[:, 4 * c:4 * c + 4],
                    tr_s.rearrange("p (j r) -> p j r", j=4)[:, :, 0],
                )
            st["rcp"] = rcp
            st["o_part"] = oparts.tile(
                [P, 4, 256], F32, tag="opart", name=f"opart{qb}_{quad}"
            )

        def t_pi(pi):
            o_part, rcp = st["o_part"], st["rcp"]
            for c in range(4):
                tr_o = ps_aux.tile([P, P], BF16, tag="aux",
                                   name=f"tro{qb}_{quad}_{pi}_{c}")
                nc.tensor.transpose(tr_o[:], st["ot_sb"][pi][:, c * P:(c + 1) * P],
                                    ident_bf[:])
                for hh in range(2):
                    lh = 2 * pi + hh
                    nc.vector.tensor_scalar(
                        o_part[:, c, 64 * lh:64 * (lh + 1)],
                        tr_o[:, 64 * hh:64 * (hh + 1)],
                        rcp[:, 4 * c + lh:4 * c + lh + 1],
                        None,
                        mybir.AluOpType.mult,
                    )

        def t_out():
            for c in range(4):
                nc.sync.dma_start(
                    out.ap()[
                        qb * 512 + c * P:qb * 512 + (c + 1) * P,
                        quad * 256:(quad + 1) * 256,
                    ],
                    st["o_part"][:, c, :],
                )

        return t0, [t1, lambda: t_pi(0), lambda: t_pi(1), t_out]

    # ---- phase 1: minimal prerequisites for the first exp ---------------
    # One DMA ring (sync), strict priority order: the critical path to the
    # first exp (wk0+st_k0+wq0+st_q0 = 3MB) streams at full bandwidth; the
    # rest of pass 1 queues behind it in need order.
    load_w("wk", wk, 0, nc.sync)
    st_k0 = stage_block(kt_r, 0, kstg, "kst", "k0", nc.sync)
    load_w("wq", wq, 0, nc.sync)
    st_q0 = stage_block(qt_r, 0, qstg, "qst", "q0", nc.sync)
    st_k1 = stage_block(kt_r, 1, kstg, "kst", "k1", nc.sync)
    load_w("wv", wv, 0, nc.sync)
    st_v0 = stage_block(vt_r, 0, vstg, "vst", "v0", nc.sync)
    kq_chunk("wk", st_k0, kproj[0][:, 0:512], 0, ps_sc)
    kq_chunk("wk", st_k0, kproj[1][:, 0:512], 1, ps_sc)
    kq_chunk("wq", st_q0, qproj[0][0][:], 0, ps_sc)
    kq_chunk("wq", st_q0, qproj[1][0][:], 1, ps_sc)
    v_chunk_half(st_v0, 0, 0)
    kstate = {0: st_k0, 1: st_k1}
    vstate = {0: st_v0}
    qstate = {0: st_q0}
    kstate[2] = stage_block(kt_r, 2, kstg, "kst", "k2", nc.sync)
    vstate[1] = stage_block(vt_r, 1, vstg, "vst", "v1", nc.sync)
    kstate[3] = stage_block(kt_r, 3, kstg, "kst", "k3", nc.sync)
    vstate[2] = stage_block(vt_r, 2, vstg, "vst", "v2", nc.sync)
    vstate[3] = stage_block(vt_r, 3, vstg, "vst", "v3", nc.sync)
    qstate[1] = stage_block(qt_r, 1, qstg, "qst", "q1", nc.sync)

    # step order: quad OUTER so pair-2/3 projections are not prerequisites
    # for the first 64 steps
    steps = [(qb, quad, k) for quad in (0, 1) for qb in range(NQB)
             for k in range(NKT)]

    # ---- filler schedule (keyed by global step index) -------------------
    fillers = {}

    def add(i, fn):
        fillers.setdefault(i, []).append(fn)

    def v_stage(kb, nm):
        def f():
            vstate[kb] = stage_block(vt_r, kb, vstg, "vst", nm, nc.sync)
        return f

    def k_stage(kb, nm):
        def f():
            kstate[kb] = stage_block(kt_r, kb, kstg, "kst", nm, nc.sync)
        return f

    def q_stage(qb, nm):
        def f():
            qstate[qb] = stage_block(qt_r, qb, qstg, "qst", nm, nc.sync)
        return f

    def mkv(kt_i, half):
        def f():
            v_chunk_half(vstate[kt_i // 4], kt_i, half)
        return f

    def mkk(kb, p, ch):
        def f():
            kq_chunk_half("wk", kstate[kb],
                          kproj[p][:, kb * 512:(kb + 1) * 512], p, ch)
        return f

    def mkq(qb, p, ch):
        def f():
            kq_chunk_half("wq", qstate[qb], qproj[p][qb][:], p, ch)
        return f

    # Fillers are ~0.87us quanta laid out against just-in-time deadlines:
    # a quantum read by AV(kt) must be emitted at step <= kt-1; one read by
    # scores(kt) of quad q at step <= 64*q + kt - 2 (scores are emitted one
    # step ahead).  Tail pieces land at unit_end + 2/4/6/8, so those steps
    # mostly stay quantum-free.
    # -- pass 1: quad-0 prerequisites (2 quanta/step: structural catch-up) --
    for kt_i in range(1, NKT):
        add(kt_i - 1, mkv(kt_i, 0))
    add(0, mkk(1, 0, 0))
    add(1, mkk(1, 1, 0))
    add(2, mkk(1, 0, 1))
    add(3, mkk(1, 1, 1))
    add(4, mkk(2, 0, 0))
    add(5, mkk(2, 1, 0))
    add(6, mkk(2, 0, 1))
    add(7, mkk(2, 1, 1))
    add(8, mkk(3, 0, 0))
    add(9, mkk(3, 1, 0))
    add(10, mkk(3, 0, 1))
    add(11, mkk(3, 1, 1))
    add(12, mkq(1, 0, 0))
    add(13, mkq(1, 0, 1))
    add(14, mkq(1, 1, 0))
    add(14, mkq(1, 1, 1))
    # -- pass 1 Q for qb2/qb3 --
    add(16, q_stage(2, "q2"))
    add(26, mkq(2, 0, 0))
    add(27, mkq(2, 0, 1))
    add(28, mkq(2, 1, 0))
    add(29, mkq(2, 1, 1))
    add(30, q_stage(3, "q3"))
    add(41, mkq(3, 0, 0))
    add(42, mkq(3, 0, 1))
    add(43, mkq(3, 1, 0))
    add(44, mkq(3, 1, 1))
    # -- pass 2 weight halves + re-staging (DMA only) --
    add(8, lambda: load_w("wk", wk, 1, nc.sync))
    add(18, lambda: load_w("wv", wv, 1, nc.sync))
    add(34, lambda: load_w("wq", wq, 1, nc.sync))
    add(32, v_stage(0, "v0b"))
    add(36, v_stage(1, "v1b"))
    add(40, k_stage(0, "k0b"))
    add(44, k_stage(1, "k1b"))
    add(44, q_stage(0, "q0b"))
    add(46, v_stage(2, "v2b"))
    add(48, k_stage(2, "k2b"))
    add(50, v_stage(3, "v3b"))
    add(52, k_stage(3, "k3b"))
    add(66, q_stage(1, "q1b"))
    add(80, q_stage(2, "q2b"))
    add(96, q_stage(3, "q3b"))
    # -- pass 2 quanta: V half-1 (deadline 63+kt), K p2/p3 (deadline
    #    62 + 4*kb + 2*ch), Q p2/p3 (deadline 63 + 16*qb) --
    add(45, mkv(0, 1))
    add(46, mkv(1, 1))
    add(47, mkv(2, 1))
    add(48, mkv(3, 1))
    add(50, mkv(4, 1))
    add(50, mkq(0, 2, 0))
    add(52, mkv(5, 1))
    add(52, mkq(0, 2, 1))
    add(54, mkv(6, 1))
    add(54, mkq(0, 3, 0))
    add(56, mkv(7, 1))
    add(56, mkq(0, 3, 1))
    add(57, mkk(0, 2, 0))
    add(57, mkv(8, 1))
    add(58, mkk(0, 3, 0))
    add(58, mkv(9, 1))
    add(59, mkk(0, 2, 1))
    add(59, mkv(10, 1))
    add(60, mkk(0, 3, 1))
    add(60, mkv(11, 1))
    add(61, mkk(1, 2, 0))
    add(61, mkv(12, 1))
    add(62, mkk(1, 3, 0))
    add(62, mkv(13, 1))
    add(63, mkk(1, 2, 1))
    add(63, mkv(14, 1))
    add(64, mkk(1, 3, 1))
    add(64, mkv(15, 1))
    add(65, mkk(2, 2, 0))
    add(66, mkk(2, 3, 0))
    add(67, mkk(2, 2, 1))
    add(68, mkk(2, 3, 1))
    add(69, mkk(3, 2, 0))
    add(70, mkk(3, 3, 0))
    add(71, mkk(3, 2, 1))
    add(72, mkk(3, 3, 1))
    add(74, mkq(1, 2, 0))
    add(75, mkq(1, 2, 1))
    add(76, mkq(1, 3, 0))
    add(77, mkq(1, 3, 1))
    add(88, mkq(2, 2, 0))
    add(89, mkq(2, 2, 1))
    add(90, mkq(2, 3, 0))
    add(91, mkq(2, 3, 1))
    add(104, mkq(3, 2, 0))
    add(105, mkq(3, 2, 1))
    add(106, mkq(3, 3, 0))
    add(107, mkq(3, 3, 1))

    # ---- the pipelined stream -------------------------------------------
    emit_scores(steps[0])
    for i, step in enumerate(steps):
        if i + 1 < len(steps):
            emit_scores(steps[i + 1])
        emit_av(step)
        qb, quad, kt_i = step
        if kt_i == NKT - 1:
            t0, rest = make_tail(qb, quad)
            t0()
            if i + 1 < len(steps):
                # spread tail PE bursts (transposes) over 8 steps so the
                # per-step PE load never exceeds the exp budget
                for j, piece in enumerate(rest):
                    add(min(i + 2 + 2 * j, len(steps) - 1), piece)
            else:
                for piece in rest:
                    piece()
        for fn in fillers.get(i, ()):
            fn()


def build():
    global _compiled
    if _compiled is not None:
        return _compiled
    nc = bacc.Bacc("TRN2", target_bir_lowering=False, debug=False)
    qt = nc.dram_tensor("qt", [P, NQB, NDT, 512], BF16, kind="ExternalInput")
    kt = nc.dram_tensor("kt", [P, NQB, NDT, 512], BF16, kind="ExternalInput")
    vt = nc.dram_tensor("vt", [P, NQB, NDT, 512], BF16, kind="ExternalInput")
    wq = nc.dram_tensor("wq", [P, 2, NDT, 256], BF16, kind="ExternalInput")
    wk = nc.dram_tensor("wk", [P, 2, NDT, 256], BF16, kind="ExternalInput")
    wv = nc.dram_tensor("wv", [P, 2, NDT, 256], BF16, kind="ExternalInput")
    bmask = nc.dram_tensor("bmask", [P, NKT], F32, kind="ExternalInput")
    out = nc.dram_tensor("out", [SEQ, CPC], F32, kind="ExternalOutput")
    with tile.TileContext(nc) as tc:
        with ExitStack() as ctx:
            _emit(ctx, tc, qt, kt, vt, wq, wk, wv, bmask, out)
    nc.compile()
    _compiled = nc
    return nc


def _sw_seq(xt):
    # [DM, SEQ] -> [P, NQB, NDT, 512]: dm = dt*128+p, q = blk*512+qq
    return np.ascontiguousarray(
        xt.reshape(NDT, P, NQB, 512).transpose(1, 2, 0, 3)
    )


def _sw_w(w):
    # [DM, CPC] -> [P, 2, NDT, 256]
    return np.ascontiguousarray(
        w.reshape(NDT, P, 2, 256).transpose(1, 2, 0, 3)
    )


def make_in_maps(Q_seq, K_seq, V_seq, V_len, WQ, WK, WV):
    in_maps = []
    for core in range(NCORES):
        b, hg = divmod(core, 2)
        cols = slice(hg * CPC, (hg + 1) * CPC)
        vl = int(V_len[b, 0])
        bm = np.ones((P, NKT), np.float32)
        bm[vl % P, vl // P] = 0.0
        vt_m = np.ascontiguousarray(V_seq[b].T)
        vt_m[:, vl] = 0.0
        bf = ml_dtypes.bfloat16
        in_maps.append(
            {
                "qt": _sw_seq(Q_seq[b].T.astype(bf)),
                "kt": _sw_seq(K_seq[b].T.astype(bf)),
                "vt": _sw_seq(vt_m.astype(bf)),
                "wq": _sw_w(WQ[:, cols].astype(bf)),
                "wk": _sw_w(WK[:, cols].astype(bf)),
                "wv": _sw_w(WV[:, cols].astype(bf)),
                "bmask": bm,
            }
        )
    return in_maps


def kernel(Q_seq, K_seq, V_seq, Q_len, V_len, WQ, WK, WV, _trace=False):
    nc = build()
    in_maps = make_in_maps(Q_seq, K_seq, V_seq, V_len, WQ, WK, WV)
    res = run_bass_kernel_spmd(
        nc, in_maps, core_ids=list(range(NCORES)), trace=_trace
    )
    out = np.empty((B, SEQ, H * DH), np.float32)
    for core in range(NCORES):
        b, hg = divmod(core, 2)
        out[b, :, hg * CPC:(hg + 1) * CPC] = res.results[core]["out"]
    for b in range(B):
        out[b, int(Q_len[b, 0]), :] = 0.0
    if _trace:
        kernel._last_results = res
    return out
